# revision 29
# baseline (speedup 1.0000x reference)
"""Trainium2 Bass kernel for capsule dynamic routing (nn_Capsule).

Reference (per batch item b):
    u = x_b @ W; logits = 0
    for i in 4:
        c = softmax(logits, axis=capsule)
        t_j = sum_s c[s,j] * u[s, j*64:(j+1)*64]; v = squash(t)
        if i < 3: logits[s,j] += u[s, jblk] . v_j

Never materializes u. By linearity (dense q = j*8 + b, 16 caps x 8 batch
= 128 partitions):
    y_q    = sum_s c[s,q] x_s          (GEMM over S, X natural layout)
    T      = y @ W  (dense q x 1024)   -> t = blockdiag(T), UNNORMALIZED,
             scattered straight into Vblk (identical layout)
    P^T    = Vblk^T W^T, then scaled by rs[q] = rsqrt(|t_q|^2 + eps)
             during the PSUM->SBUF copy (squash deferred off the PE path)
    upd    = X P via X^T (fp8 operands; logits only steer routing)
The squash norm is computed from Vblk^2 with a PE partition-reduce
issued ahead of the P^T matmuls; rsqrt is a quake-style bit trick + 2
Newton steps on DVE (Scalar stays on one act table: copy/exp only).

HW lessons encoded:
  - ScalarE activation(Copy) for every PSUM f32 -> f16 cast (DVE dies).
  - Each PE-transpose output gets its own PSUM tile.
  - Engine APs need 32-aligned partition bases on PSUM; DMA cannot
    touch PSUM; tensor_tensor_reduce faults the device.
  - matmul start=True lazily zeroes the PSUM bank for the out AP's
    partitions; partition-disjoint groups use skip_group_check=True.
  - f16 constants come from host DRAM, not memset.
  - Every dma_start costs ~660ns of serial sequencer time (DIRECT2D);
    with 16 HW queues at ~1/16 bandwidth each, use ~0.5-1MB chunks and
    split the triggers across both HWDGE engines (sync + scalar).
"""
import numpy as np
import ml_dtypes
from contextlib import ExitStack

import concourse.bass as bass
import concourse.bacc as bacc
import concourse.tile as tile
from concourse import mybir
from concourse.bass_utils import run_bass_kernel_spmd

f16 = mybir.dt.float16
f32 = mybir.dt.float32
f8 = mybir.dt.float8e4
i32 = mybir.dt.int32
COPY = mybir.ActivationFunctionType.Copy
EXP = mybir.ActivationFunctionType.Exp

S, B, H = 512, 64, 1024
NCAP, DCAP = 16, 64
ROUTINGS = 4
N_CORES = 8
BL = B // N_CORES          # 8 batch items per core
SC = S // 128              # 4 s-chunks
HC = H // 128              # 8 h-chunks
OC = H // 128              # 8 o-chunks (o = NCAP*DCAP = 1024)

# f16 const blob column offsets: id16 | onescol  (read-only consts)
C_ID16 = 0
C_ONE = 128
NC16 = C_ONE + 1
# f32 const blob: id32 | ones1(row0)
C_ID32 = 0
C_ONES1 = 128
NC32 = C_ONES1 + 1


def _act_copy(nc, out, in_, scale=1.0):
    nc.scalar.activation(out=out, in_=in_, func=COPY, scale=scale, alpha=0.0)


def _quake_rsqrt(nc, small, n_ap, eps_val, shape, it):
    """rs = 1/sqrt(n + eps) on DVE (bit trick + 2 Newton steps).
    n_ap may live in PSUM (f32). Returns an SBUF f32 AP of `shape`."""
    nn_ = small.tile(shape, f32, tag="qk_nn", name=f"nn{it}")
    nc.vector.tensor_scalar_add(nn_[:], n_ap, eps_val)
    sh = small.tile(shape, i32, tag="qk_sh", name=f"sh{it}")
    nc.vector.tensor_scalar(
        out=sh[:], in0=nn_.bitcast(i32), scalar1=1, scalar2=None,
        op0=mybir.AluOpType.arith_shift_right)
    r0i = small.tile(shape, i32, tag="qk_r0", name=f"r0i{it}")
    nc.vector.tensor_scalar(
        out=r0i[:], in0=sh[:], scalar1=-1, scalar2=0x5F3759DF,
        op0=mybir.AluOpType.mult, op1=mybir.AluOpType.add)
    rprev = r0i.bitcast(f32)
    for newt in range(2):
        ra = small.tile(shape, f32, tag="qk_ra", name=f"ra{it}{newt}")
        nc.vector.tensor_mul(ra[:], rprev, rprev)
        rb = small.tile(shape, f32, tag="qk_rb", name=f"rb{it}{newt}")
        nc.vector.scalar_tensor_tensor(
            out=rb[:], in0=ra[:], scalar=-0.5, in1=nn_[:],
            op0=mybir.AluOpType.mult, op1=mybir.AluOpType.mult)
        rd = small.tile(shape, f32, tag="qk_rd", name=f"rd{it}{newt}")
        nc.vector.scalar_tensor_tensor(
            out=rd[:], in0=rb[:], scalar=1.5, in1=rprev,
            op0=mybir.AluOpType.add, op1=mybir.AluOpType.mult)
        rprev = rd[:]
    return rprev


def _build_kernel(tc, out_d, x_d, xt_d, w_d, cb16_d, cb32_d, scr_d,
                  cpad_d, logits_d, vblk_d):
    nc = tc.nc
    ctx = ExitStack()
    const = ctx.enter_context(tc.tile_pool(name="const", bufs=1))
    work = ctx.enter_context(tc.tile_pool(name="work", bufs=1))
    small = ctx.enter_context(tc.tile_pool(name="small", bufs=2))
    ps_big = ctx.enter_context(tc.tile_pool(name="ps_big", bufs=1,
                                            space="PSUM"))
    ps_tp = ctx.enter_context(tc.tile_pool(name="ps_tp", bufs=4, space="PSUM"))

    # ---------- persistent tensors ----------
    x16 = const.tile([128, BL, SC, 1024], f16)    # X natural (s_loc, b, sc, h)
    xt8 = const.tile([128, BL, HC, 512], f16)     # X^T (h_loc, b, hc, s)
    w16 = const.tile([128, HC, 1024], f16)        # W natural (h_loc, hc, o)
    wt16 = const.tile([128, OC, 1024], f16)       # W^T (derived on-chip)
    cb16 = const.tile([128, NC16], f16)           # f16 const blob
    cb32 = const.tile([128, NC32], f32)           # f32 const blob
    id16 = cb16[:, C_ID16:C_ID16 + 128]
    onescol = cb16[:, C_ONE:C_ONE + 1]
    id32 = cb32[:, C_ID32:C_ID32 + 128]
    ones1 = cb32[0:1, C_ONES1:C_ONES1 + 1]
    cpad = const.tile([128, BL, SC, 32], f16)     # c, cols 16-31 zero pad
    logits = const.tile([128, BL, SC, 16], f32)
    vblk = const.tile([128, OC, 128], f16)        # block-diag t, dense q

    # ---------- loads: 2 const blobs, then chunks by first use ----------
    # Alternate trigger engines (sync/scalar) - each dma_start costs
    # ~660ns of serial sequencer time.
    engs = [nc.sync, nc.scalar]
    tcount = [0]

    def dma(out, in_):
        engs[tcount[0] % 2].dma_start(out=out, in_=in_)
        tcount[0] += 1

    dma(cb16[:], cb16_d[:])
    dma(cb32[:], cb32_d[:])
    dma(cpad[:], cpad_d[:])
    dma(logits[:], logits_d[:])
    dma(vblk[:], vblk_d[:])
    for hc in range(0, HC, 2):                  # w16 first (derives ride)
        dma(w16[:, hc:hc + 2, :], w_d[:, hc:hc + 2, :])
    for sc in range(SC):
        for half in range(2):
            sl = slice(512 * half, 512 * half + 512)
            dma(x16[:, :, sc, sl], x_d[:, :, sc, sl])
    for b in range(4):                          # xt b0-3 only; b4-7 derived
        for hh in range(0, HC, 4):
            dma(xt8[:, b, hh:hh + 4, :], xt_d[:, b, hh:hh + 4, :])

    # ---------- derive W^T and xt(b4-7) on-chip, riding the load ------
    # Emitted BEFORE the loop so the PE fills the DMA window; data is
    # bit-identical to the host-packed version (same f16 transposes).
    for hc in range(HC):
        for oc in range(OC):
            tpw = ps_tp.tile([128, 128], f16, tag="tp",
                             name=f"w_tp{hc}_{oc}")
            nc.tensor.matmul(tpw[:], w16[:, hc, 128 * oc:128 * oc + 128],
                             id16[:], is_transpose=True,
                             skip_group_check=True)
            nc.vector.tensor_copy(wt16[:, oc, 128 * hc:128 * hc + 128],
                                  tpw[:])
    for sc in range(SC):
        for b in range(4, BL):
            for hc in range(HC):
                tpx = ps_tp.tile([128, 128], f16, tag="tp",
                                 name=f"x_tp{sc}_{b}_{hc}")
                nc.tensor.matmul(
                    tpx[:], x16[:, b, sc, 128 * hc:128 * hc + 128],
                    id16[:], is_transpose=True, skip_group_check=True)
                nc.vector.tensor_copy(
                    xt8[:, b, hc, 128 * sc:128 * sc + 128], tpx[:])

    v32 = None
    for it in range(ROUTINGS):
        last = it == ROUTINGS - 1

        # ---------- y = C^T X: (2g x 4b x 32pad part, 1024 h) f32 --------
        y_ps = ps_big.tile([128, 2, 1024], f32, tag="big", name=f"y_ps{it}")
        for g in range(2):
            for sc in range(SC):
                for b_ in range(4):
                    b = 4 * g + b_
                    for half in range(2):   # same weights both halves
                        nc.tensor.matmul(
                            y_ps[32 * b_:32 * b_ + 32, g,
                                 512 * half:512 * half + 512],
                            cpad[:, b, sc, :],
                            x16[:, b, sc, 512 * half:512 * half + 512],
                            start=(sc == 0), stop=(sc == SC - 1),
                            skip_group_check=True,
                            tile_position=(0, 32 * b_))
        y_sb = work.tile([128, 2, 1024], f16, tag="y_sb")
        for g in range(2):
            for half in range(2):
                _act_copy(nc, y_sb[:, g, 512 * half:512 * half + 512],
                          y_ps[:, g, 512 * half:512 * half + 512])

        # ---------- y^T via PE transposes, dense capsule-major pack ------
        yt = work.tile([128, HC, 128], f16, tag="yt")
        for hc in range(HC):
            for g in range(2):
                tp = ps_tp.tile([128, 128], f16, tag="tp",
                                name=f"yt_tp{it}_{hc}_{g}")
                nc.tensor.matmul(
                    tp[:], y_sb[:, g, 128 * hc:128 * hc + 128], id16[:],
                    is_transpose=True, skip_group_check=True)
                src = tp.rearrange("p (b j) -> p j b", j=32)[:, 0:16, :]
                dst = yt[:, hc, :].rearrange(
                    "p (j b) -> p j b", b=8)[:, :, 4 * g:4 * g + 4]
                nc.vector.tensor_copy(dst, src)

        # ---------- T = y W: (128 q, 1024 o) f32, dense ----------
        t_ps = ps_big.tile([128, 1024], f32, tag="big", name=f"t_ps{it}")
        for half in range(2):
            for hc in range(HC):
                nc.tensor.matmul(
                    t_ps[:, 512 * half:512 * half + 512],
                    yt[:, hc, :],
                    w16[:, hc, 512 * half:512 * half + 512],
                    start=(hc == 0), stop=(hc == HC - 1))
        t_sb16 = work.tile([128, 1024], f16, tag="t_sb16")
        for half in range(2):
            _act_copy(nc, t_sb16[:, 512 * half:512 * half + 512],
                      t_ps[:, 512 * half:512 * half + 512])

        if last:
            # ---------- dense t via 16 half-width transposes ----------
            # ttd[d, q] = t[q, d] for q in [8j, 8j+8) - d dense for ALL j.
            ttd = work.tile([64, 128], f16, tag="ttd")
            for j in range(NCAP):
                tpj = ps_tp.tile([64, 128], f16, tag="tp",
                                 name=f"td_tp{j}")
                nc.tensor.matmul(tpj[:], t_sb16[:, 64 * j:64 * j + 64],
                                 id16[:], is_transpose=True,
                                 skip_group_check=True)
                nc.vector.tensor_copy(ttd[0:64, 8 * j:8 * j + 8],
                                      tpj[0:64, 8 * j:8 * j + 8])
            ttq = ps_tp.tile([128, 64], f16, tag="tp", name="ttq3")
            nc.tensor.matmul(ttq[:], ttd[:], id16[0:64, 0:64],
                             is_transpose=True, skip_group_check=True)
            t_sb = small.tile([128, 64], f16, tag="t_sb")
            nc.vector.tensor_copy(t_sb[:], ttq[:])
            t2 = small.tile([128, 64], f32, tag="t2")
            nc.vector.tensor_mul(t2[:], t_sb[:], t_sb[:])
            ssum = small.tile([128, 1], f32, tag="ssum")
            nc.vector.reduce_sum(ssum[:], t2[:], axis=mybir.AxisListType.X)
            rs = _quake_rsqrt(nc, small, ssum[:], 1e-7, [128, 1], it)
            v32 = small.tile([128, 64], f32, tag="v32")
            nc.vector.tensor_mul(v32[:], t_sb[:], rs.broadcast_to([128, 64]))
            break

        # ---------- T^T chunks via PE transpose; t -> vblk directly ------
        for oc in range(OC):
            ttp = ps_tp.tile([128, 128], f16, tag="tp",
                             name=f"tt_tp{it}_{oc}")
            nc.tensor.matmul(ttp[:], t_sb16[:, 128 * oc:128 * oc + 128],
                             id16[:], is_transpose=True,
                             skip_group_check=True)
            for par in range(2):
                j = 2 * oc + par
                p0 = 64 * par
                nc.vector.tensor_copy(
                    vblk[p0:p0 + 64, oc, 8 * j:8 * j + 8],
                    ttp[p0:p0 + 64, 8 * j:8 * j + 8])

        # ---------- norm from vblk^2 (gpsimd; overlaps extracts) --------
        sq = work.tile([128, OC, 128], f16, tag="sq")
        for oc in range(OC):
            nc.gpsimd.tensor_mul(sq[:, oc, :], vblk[:, oc, :],
                                 vblk[:, oc, :])

        # ---------- norm reduce ahead of P^T; then P^T = Vblk^T W^T -----
        n_ps = ps_tp.tile([1, 128], f32, tag="tp", name=f"n_ps{it}")
        for oc in range(OC):
            nc.tensor.matmul(
                n_ps[:], onescol[:], sq[:, oc, :],
                start=(oc == 0), stop=(oc == OC - 1))
        pt_ps = ps_big.tile([128, 1024], f32, tag="big", name=f"pt_ps{it}")
        for half in range(2):
            for oc in range(OC):
                nc.tensor.matmul(
                    pt_ps[:, 512 * half:512 * half + 512],
                    vblk[:, oc, :],
                    wt16[:, oc, 512 * half:512 * half + 512],
                    start=(oc == 0), stop=(oc == OC - 1))
        rs_row = _quake_rsqrt(nc, small, n_ps[:], 1e-7, [1, 128], it)
        # broadcast rs_row (1,128) -> column (128,1) via 1-contraction MM
        rs_ps = ps_tp.tile([128, 1], f32, tag="tp", name=f"rs_ps{it}")
        nc.tensor.matmul(rs_ps[:], rs_row, ones1,
                         skip_group_check=True)
        rs_col = small.tile([128, 1], f32, tag="rs_col")
        nc.vector.tensor_copy(rs_col[:], rs_ps[:])

        # ---------- pt_sb = rs[q] * P^T  (squash applied here) ----------
        pt_sb = work.tile([128, 1024], f16, tag="pt_sb")
        for qtr in range(4):
            sl = slice(256 * qtr, 256 * qtr + 256)
            nc.scalar.activation(out=pt_sb[:, sl], in_=pt_ps[:, sl],
                                 func=COPY, scale=rs_col[:], alpha=0.0)

        # ---------- transpose P^T -> P natural; cast fp8 ----------
        p8 = work.tile([128, HC, 128], f16, tag="p8")
        for hc in range(HC):
            ptp = ps_tp.tile([128, 128], f16, tag="tp",
                             name=f"p_tp{it}_{hc}")
            nc.tensor.matmul(
                ptp[:], pt_sb[:, 128 * hc:128 * hc + 128], id16[:],
                is_transpose=True, skip_group_check=True)
            nc.vector.tensor_copy(p8[:, hc, :], ptp[:])

        # ---------- update = X P via X^T (fp8, strided 16-col weights) ---
        u_ps = ps_big.tile([128, 2, 512], f32, tag="big", name=f"u_ps{it}")
        for g in range(2):
            for hc in range(HC):
                for b_ in range(4):
                    b = 4 * g + b_
                    wcols = p8[:, hc, :].rearrange(
                        "p (j b) -> p b j", b=8)[:, b, :]
                    nc.tensor.matmul(
                        u_ps[32 * b_:32 * b_ + 16, g, :],
                        wcols,
                        xt8[:, b, hc, :],
                        start=(hc == 0), stop=(hc == HC - 1),
                        skip_group_check=True,
                        tile_position=(0, 32 * b_))
        u_sb = work.tile([128, 2, 512], f32, tag="u_sb")
        for g in range(2):
            nc.vector.tensor_copy(u_sb[:, g, :], u_ps[:, g, :])

        # ---------- transpose update, accumulate logits ----------
        for sc in range(SC):
            for g in range(2):
                utp = ps_tp.tile([128, 128], f32, tag="tp",
                                 name=f"ut_tp{it}_{sc}_{g}")
                nc.tensor.matmul(
                    utp[:], u_sb[:, g, 128 * sc:128 * sc + 128], id32[:],
                    is_transpose=True, skip_group_check=True)
                src = utp.rearrange("p (b j) -> p b j", j=32)
                nc.vector.tensor_add(
                    logits[:, 4 * g:4 * g + 4, sc, :],
                    logits[:, 4 * g:4 * g + 4, sc, :], src[:, :, 0:16])

        # ---------- softmax over capsules -> cpad ----------
        for sc in range(SC):
            ex = small.tile([128, BL, 16], f32, tag="ex")
            nc.scalar.activation(out=ex[:], in_=logits[:, :, sc, :],
                                 func=EXP, scale=1.0, alpha=0.0)
            sm = small.tile([128, BL, 1], f32, tag="sm")
            nc.vector.reduce_sum(sm[:], ex[:], axis=mybir.AxisListType.X)
            rc = small.tile([128, BL, 1], f32, tag="rc")
            nc.vector.reciprocal(rc[:], sm[:])
            nc.vector.tensor_mul(cpad[:, :, sc, 0:16], ex[:],
                                 rc.broadcast_to([128, BL, 16]))

    # ---------- out[b, j, d] = v32[q = j*8 + b, d] via DRAM gather ------
    nc.sync.dma_start(out=scr_d[:], in_=v32[:])
    gather_in = bass.AP(tensor=scr_d.tensor, offset=0,
                        ap=[[512, 16], [64, 8], [1, 64]])
    gather_out = bass.AP(tensor=out_d.tensor, offset=0,
                         ap=[[64, 16], [1024, 8], [1, 64]])
    nc.scalar.dma_start(out=gather_out, in_=gather_in)
    ctx.close()


_CACHE = {}


def _host_consts():
    cb16 = np.zeros((128, NC16), np.float16)
    cb16[:, C_ID16:C_ID16 + 128] = np.eye(128, dtype=np.float16)
    cb16[:, C_ONE] = 1.0                      # onescol
    cb32 = np.zeros((128, NC32), np.float32)
    cb32[:, C_ID32:C_ID32 + 128] = np.eye(128, dtype=np.float32)
    cb32[0, C_ONES1] = 1.0
    cpad = np.zeros((128, BL, SC, 32), np.float16)
    cpad[:, :, :, 0:16] = 1.0 / NCAP          # iteration-0 softmax is exact
    logi = np.zeros((128, BL, SC, 16), np.float32)
    vblk = np.zeros((128, OC, 128), np.float16)
    return {"cb16": cb16, "cb32": cb32, "cpadi": cpad, "logi": logi,
            "vblki": vblk}


def _get_nc():
    if "nc" not in _CACHE:
        nc = bacc.Bacc("TRN2", target_bir_lowering=False, debug=False)
        x_d = nc.dram_tensor("xh", [128, BL, SC, 1024], f16,
                             kind="ExternalInput")
        xt_d = nc.dram_tensor("xth", [128, BL, HC, 512], f16,
                              kind="ExternalInput")
        w_d = nc.dram_tensor("wh", [128, HC, 1024], f16,
                             kind="ExternalInput")
        cb16_d = nc.dram_tensor("cb16", [128, NC16], f16,
                                kind="ExternalInput")
        cb32_d = nc.dram_tensor("cb32", [128, NC32], f32,
                                kind="ExternalInput")
        cpad_d = nc.dram_tensor("cpadi", [128, BL, SC, 32], f16,
                                kind="ExternalInput")
        logits_d = nc.dram_tensor("logi", [128, BL, SC, 16], f32,
                                  kind="ExternalInput")
        vblk_d = nc.dram_tensor("vblki", [128, OC, 128], f16,
                                kind="ExternalInput")
        scr_d = nc.dram_tensor("scratch", [128, 64], f32, kind="Internal")
        out_d = nc.dram_tensor("out", [BL, NCAP, DCAP], f32,
                               kind="ExternalOutput")
        with tile.TileContext(nc) as tc:
            _build_kernel(tc, out_d.ap(), x_d.ap(), xt_d.ap(), w_d.ap(),
                          cb16_d.ap(), cb32_d.ap(), scr_d.ap(),
                          cpad_d.ap(), logits_d.ap(), vblk_d.ap())
        nc.compile()
        _CACHE["nc"] = nc
    return _CACHE["nc"]


def kernel(inputs: np.ndarray, W: np.ndarray, _trace: bool = False):
    """inputs: (512, 64, 1024) f32; W: (1, 1024, 1024) f32.
    Returns (64, 16, 64) f32."""
    nc = _get_nc()
    consts = _host_consts()
    w0 = W[0].astype(np.float16)
    wh = np.ascontiguousarray(w0.reshape(HC, 128, 1024).transpose(1, 0, 2))
    xf = inputs.astype(np.float16)              # (512, 64, 1024)
    in_maps = []
    for c in range(N_CORES):
        xs = xf[:, c * BL:(c + 1) * BL, :]      # (512, BL, 1024)
        xh = np.ascontiguousarray(
            xs.reshape(SC, 128, BL, 1024).transpose(1, 2, 0, 3))
        xth = np.ascontiguousarray(
            xs.reshape(512, BL, HC, 128).transpose(3, 1, 2, 0))
        m = {"xh": xh, "xth": xth, "wh": wh}
        m.update(consts)
        in_maps.append(m)
    kw = {}
    if _trace:
        kw = dict(trace=True, trace_cores=list(range(N_CORES)),
                  stitch_traces=False)
    res = run_bass_kernel_spmd(nc, in_maps, core_ids=list(range(N_CORES)),
                               **kw)
    out = np.concatenate([res.results[c]["out"] for c in range(N_CORES)],
                         axis=0)
    if _trace:
        return out.astype(np.float32), res
    return out.astype(np.float32)


# revision 31
# speedup vs baseline: 1.2191x; 1.2191x over previous
"""Trainium2 Bass kernel for capsule dynamic routing (nn_Capsule).

Reference (per batch item b):
    u = x_b @ W; logits = 0
    for i in 4:
        c = softmax(logits, axis=capsule)
        t_j = sum_s c[s,j] * u[s, j*64:(j+1)*64]; v = squash(t)
        if i < 3: logits[s,j] += u[s, jblk] . v_j

Never materializes u. By linearity (dense q = j*8 + b, 16 caps x 8 batch
= 128 partitions):
    y_q    = sum_s c[s,q] x_s          (GEMM over S, X natural layout)
    T      = y @ W  (dense q x 1024)   -> t = blockdiag(T), UNNORMALIZED,
             scattered straight into Vblk (identical layout)
    P^T    = Vblk^T W^T, then scaled by rs[q] = rsqrt(|t_q|^2 + eps)
             during the PSUM->SBUF copy (squash deferred off the PE path)
    upd    = X P via X^T (fp8 operands; logits only steer routing)
The squash norm is computed from Vblk^2 with a PE partition-reduce
issued ahead of the P^T matmuls; rsqrt is a quake-style bit trick + 2
Newton steps on DVE (Scalar stays on one act table: copy/exp only).

HW lessons encoded:
  - ScalarE activation(Copy) for every PSUM f32 -> f16 cast (DVE dies).
  - Each PE-transpose output gets its own PSUM tile.
  - Engine APs need 32-aligned partition bases on PSUM; DMA cannot
    touch PSUM; tensor_tensor_reduce faults the device.
  - matmul start=True lazily zeroes the PSUM bank for the out AP's
    partitions; partition-disjoint groups use skip_group_check=True.
  - f16 constants come from host DRAM, not memset.
  - Every dma_start costs ~660ns of serial sequencer time (DIRECT2D);
    with 16 HW queues at ~1/16 bandwidth each, use ~0.5-1MB chunks and
    split the triggers across both HWDGE engines (sync + scalar).
"""
import numpy as np
import ml_dtypes
from contextlib import ExitStack

import concourse.bass as bass
import concourse.bacc as bacc
import concourse.tile as tile
from concourse import mybir
from concourse.bass_utils import run_bass_kernel_spmd

f16 = mybir.dt.float16
f32 = mybir.dt.float32
f8 = mybir.dt.float8e4
i32 = mybir.dt.int32
COPY = mybir.ActivationFunctionType.Copy
EXP = mybir.ActivationFunctionType.Exp

S, B, H = 512, 64, 1024
NCAP, DCAP = 16, 64
ROUTINGS = 4
N_CORES = 8
BL = B // N_CORES          # 8 batch items per core
SC = S // 128              # 4 s-chunks
HC = H // 128              # 8 h-chunks
OC = H // 128              # 8 o-chunks (o = NCAP*DCAP = 1024)

# f16 const blob column offsets: id16 | onescol  (read-only consts)
C_ID16 = 0
C_ONE = 128
NC16 = C_ONE + 1
# f32 const blob: id32 | ones1(row0)
C_ID32 = 0
C_ONES1 = 128
NC32 = C_ONES1 + 1


def _act_copy(nc, out, in_, scale=1.0):
    nc.scalar.activation(out=out, in_=in_, func=COPY, scale=scale, alpha=0.0)


def _quake_rsqrt(nc, small, n_ap, eps_val, shape, it):
    """rs = 1/sqrt(n + eps) on DVE (bit trick + 2 Newton steps).
    n_ap may live in PSUM (f32). Returns an SBUF f32 AP of `shape`."""
    nn_ = small.tile(shape, f32, tag="qk_nn", name=f"nn{it}")
    nc.vector.tensor_scalar_add(nn_[:], n_ap, eps_val)
    sh = small.tile(shape, i32, tag="qk_sh", name=f"sh{it}")
    nc.vector.tensor_scalar(
        out=sh[:], in0=nn_.bitcast(i32), scalar1=1, scalar2=None,
        op0=mybir.AluOpType.arith_shift_right)
    r0i = small.tile(shape, i32, tag="qk_r0", name=f"r0i{it}")
    nc.vector.tensor_scalar(
        out=r0i[:], in0=sh[:], scalar1=-1, scalar2=0x5F3759DF,
        op0=mybir.AluOpType.mult, op1=mybir.AluOpType.add)
    rprev = r0i.bitcast(f32)
    for newt in range(2):
        ra = small.tile(shape, f32, tag="qk_ra", name=f"ra{it}{newt}")
        nc.vector.tensor_mul(ra[:], rprev, rprev)
        rb = small.tile(shape, f32, tag="qk_rb", name=f"rb{it}{newt}")
        nc.vector.scalar_tensor_tensor(
            out=rb[:], in0=ra[:], scalar=-0.5, in1=nn_[:],
            op0=mybir.AluOpType.mult, op1=mybir.AluOpType.mult)
        rd = small.tile(shape, f32, tag="qk_rd", name=f"rd{it}{newt}")
        nc.vector.scalar_tensor_tensor(
            out=rd[:], in0=rb[:], scalar=1.5, in1=rprev,
            op0=mybir.AluOpType.add, op1=mybir.AluOpType.mult)
        rprev = rd[:]
    return rprev


def _build_kernel(tc, out_d, x_d, xt_d, w_d, cb16_d, cb32_d, scr_d,
                  cpad_d, logits_d, vblk_d):
    nc = tc.nc
    ctx = ExitStack()
    const = ctx.enter_context(tc.tile_pool(name="const", bufs=1))
    work = ctx.enter_context(tc.tile_pool(name="work", bufs=1))
    small = ctx.enter_context(tc.tile_pool(name="small", bufs=2))
    ps_big = ctx.enter_context(tc.tile_pool(name="ps_big", bufs=1,
                                            space="PSUM"))
    ps_tp = ctx.enter_context(tc.tile_pool(name="ps_tp", bufs=4, space="PSUM"))

    # ---------- persistent tensors ----------
    x16 = const.tile([128, BL, SC, 1024], f16)    # X natural (s_loc, b, sc, h)
    xt8 = const.tile([128, BL, HC, 512], f16)     # X^T (h_loc, b, hc, s)
    w16 = const.tile([128, HC, 1024], f16)        # W natural (h_loc, hc, o)
    wt16 = const.tile([128, OC, 1024], f16)       # W^T (derived on-chip)
    cb16 = const.tile([128, NC16], f16)           # f16 const blob
    cb32 = const.tile([128, NC32], f32)           # f32 const blob
    id16 = cb16[:, C_ID16:C_ID16 + 128]
    onescol = cb16[:, C_ONE:C_ONE + 1]
    id32 = cb32[:, C_ID32:C_ID32 + 128]
    ones1 = cb32[0:1, C_ONES1:C_ONES1 + 1]
    cpad = const.tile([128, BL, SC, 32], f16)     # c, cols 16-31 zero pad
    logits = const.tile([128, BL, SC, 16], f32)
    vblk = const.tile([128, OC, 128], f16)        # block-diag t, dense q

    # ---------- loads: 2 const blobs, then chunks by first use ----------
    # Alternate trigger engines (sync/scalar) - each dma_start costs
    # ~660ns of serial sequencer time.
    engs = [nc.sync, nc.scalar]
    tcount = [0]

    def dma(out, in_):
        engs[tcount[0] % 2].dma_start(out=out, in_=in_)
        tcount[0] += 1

    dma(cb16[:], cb16_d[:])
    dma(cb32[:], cb32_d[:])
    dma(cpad[:], cpad_d[:])
    dma(logits[:], logits_d[:])
    dma(vblk[:], vblk_d[:])
    for half in range(2):                       # x sc0 first (y0 rides)
        sl = slice(512 * half, 512 * half + 512)
        dma(x16[:, :, 0, sl], x_d[:, :, 0, sl])
    for hc in range(0, HC, 2):                  # w16 in 4 chunks
        dma(w16[:, hc:hc + 2, :], w_d[:, hc:hc + 2, :])
    for sc in range(1, SC):
        for half in range(2):
            sl = slice(512 * half, 512 * half + 512)
            dma(x16[:, :, sc, sl], x_d[:, :, sc, sl])
    for b in range(BL):
        for hh in range(0, HC, 4):              # xt in 16 chunks
            dma(xt8[:, b, hh:hh + 4, :], xt_d[:, b, hh:hh + 4, :])

    v32 = None
    for it in range(ROUTINGS):
        last = it == ROUTINGS - 1

        # ---------- y = C^T X: (2g x 4b x 32pad part, 1024 h) f32 --------
        y_ps = ps_big.tile([128, 2, 1024], f32, tag="big", name=f"y_ps{it}")
        for sc in range(SC):
            for g in range(2):
                for b_ in range(4):
                    b = 4 * g + b_
                    for half in range(2):   # same weights both halves
                        nc.tensor.matmul(
                            y_ps[32 * b_:32 * b_ + 32, g,
                                 512 * half:512 * half + 512],
                            cpad[:, b, sc, :],
                            x16[:, b, sc, 512 * half:512 * half + 512],
                            start=(sc == 0), stop=(sc == SC - 1),
                            skip_group_check=True,
                            tile_position=(0, 32 * b_))
        y_sb = work.tile([128, 2, 1024], f16, tag="y_sb")
        for g in range(2):
            for qt in range(4):
                _act_copy(nc, y_sb[:, g, 256 * qt:256 * qt + 256],
                          y_ps[:, g, 256 * qt:256 * qt + 256])

        # ---------- y^T via PE transposes, dense capsule-major pack ------
        yt = work.tile([128, HC, 128], f16, tag="yt")
        for hc in range(HC):
            for g in range(2):
                tp = ps_tp.tile([128, 128], f16, tag="tp",
                                name=f"yt_tp{it}_{hc}_{g}")
                nc.tensor.matmul(
                    tp[:], y_sb[:, g, 128 * hc:128 * hc + 128], id16[:],
                    is_transpose=True, skip_group_check=True)
                src = tp.rearrange("p (b j) -> p j b", j=32)[:, 0:16, :]
                dst = yt[:, hc, :].rearrange(
                    "p (j b) -> p j b", b=8)[:, :, 4 * g:4 * g + 4]
                nc.vector.tensor_copy(dst, src)

        if it == 0:
            # ---------- derive W^T on-chip (hides under the xt load) -----
            for hc in range(HC):
                for oc in range(OC):
                    tpw = ps_tp.tile([128, 128], f16, tag="tp",
                                     name=f"w_tp{hc}_{oc}")
                    nc.tensor.matmul(
                        tpw[:], w16[:, hc, 128 * oc:128 * oc + 128],
                        id16[:], is_transpose=True, skip_group_check=True)
                    nc.vector.tensor_copy(
                        wt16[:, oc, 128 * hc:128 * hc + 128], tpw[:])

        # ---------- T = y W: (128 q, 1024 o) f32, dense ----------
        t_ps = ps_big.tile([128, 1024], f32, tag="big", name=f"t_ps{it}")
        for half in range(2):
            for hc in range(HC):
                nc.tensor.matmul(
                    t_ps[:, 512 * half:512 * half + 512],
                    yt[:, hc, :],
                    w16[:, hc, 512 * half:512 * half + 512],
                    start=(hc == 0), stop=(hc == HC - 1))
        t_sb16 = work.tile([128, 1024], f16, tag="t_sb16")
        for qt in range(4):
            _act_copy(nc, t_sb16[:, 256 * qt:256 * qt + 256],
                      t_ps[:, 256 * qt:256 * qt + 256])

        if last:
            # ---------- dense t via 16 half-width transposes ----------
            # ttd[d, q] = t[q, d] for q in [8j, 8j+8) - d dense for ALL j.
            ttd = work.tile([64, 128], f16, tag="ttd")
            for j in range(NCAP):
                tpj = ps_tp.tile([64, 128], f16, tag="tp",
                                 name=f"td_tp{j}")
                nc.tensor.matmul(tpj[:], t_sb16[:, 64 * j:64 * j + 64],
                                 id16[:], is_transpose=True,
                                 skip_group_check=True)
                nc.vector.tensor_copy(ttd[0:64, 8 * j:8 * j + 8],
                                      tpj[0:64, 8 * j:8 * j + 8])
            ttq = ps_tp.tile([128, 64], f16, tag="tp", name="ttq3")
            nc.tensor.matmul(ttq[:], ttd[:], id16[0:64, 0:64],
                             is_transpose=True, skip_group_check=True)
            t_sb = small.tile([128, 64], f16, tag="t_sb")
            nc.vector.tensor_copy(t_sb[:], ttq[:])
            t2 = small.tile([128, 64], f32, tag="t2")
            nc.vector.tensor_mul(t2[:], t_sb[:], t_sb[:])
            ssum = small.tile([128, 1], f32, tag="ssum")
            nc.vector.reduce_sum(ssum[:], t2[:], axis=mybir.AxisListType.X)
            rs = _quake_rsqrt(nc, small, ssum[:], 1e-7, [128, 1], it)
            v32 = small.tile([128, 64], f32, tag="v32")
            nc.vector.tensor_mul(v32[:], t_sb[:], rs.broadcast_to([128, 64]))
            break

        # ---------- T^T chunks via PE transpose; t -> vblk directly ------
        for oc in range(OC):
            ttp = ps_tp.tile([128, 128], f16, tag="tp",
                             name=f"tt_tp{it}_{oc}")
            nc.tensor.matmul(ttp[:], t_sb16[:, 128 * oc:128 * oc + 128],
                             id16[:], is_transpose=True,
                             skip_group_check=True)
            for par in range(2):
                j = 2 * oc + par
                p0 = 64 * par
                nc.vector.tensor_copy(
                    vblk[p0:p0 + 64, oc, 8 * j:8 * j + 8],
                    ttp[p0:p0 + 64, 8 * j:8 * j + 8])

        # ---------- norm from vblk^2 (gpsimd; overlaps extracts) --------
        sq = work.tile([128, OC, 128], f16, tag="sq")
        for oc in range(OC):
            nc.gpsimd.tensor_mul(sq[:, oc, :], vblk[:, oc, :],
                                 vblk[:, oc, :])

        # ---------- norm reduce ahead of P^T; then P^T = Vblk^T W^T -----
        n_ps = ps_tp.tile([1, 128], f32, tag="tp", name=f"n_ps{it}")
        for oc in range(OC):
            nc.tensor.matmul(
                n_ps[:], onescol[:], sq[:, oc, :],
                start=(oc == 0), stop=(oc == OC - 1))
        pt_ps = ps_big.tile([128, 1024], f32, tag="big", name=f"pt_ps{it}")
        for half in range(2):
            for oc in range(OC):
                nc.tensor.matmul(
                    pt_ps[:, 512 * half:512 * half + 512],
                    vblk[:, oc, :],
                    wt16[:, oc, 512 * half:512 * half + 512],
                    start=(oc == 0), stop=(oc == OC - 1))
        rs_row = _quake_rsqrt(nc, small, n_ps[:], 1e-7, [1, 128], it)
        # broadcast rs_row (1,128) -> column (128,1) via 1-contraction MM
        rs_ps = ps_tp.tile([128, 1], f32, tag="tp", name=f"rs_ps{it}")
        nc.tensor.matmul(rs_ps[:], rs_row, ones1,
                         skip_group_check=True)
        rs_col = small.tile([128, 1], f32, tag="rs_col")
        nc.vector.tensor_copy(rs_col[:], rs_ps[:])

        # ---------- pt_sb = rs[q] * P^T  (squash applied here) ----------
        pt_sb = work.tile([128, 1024], f16, tag="pt_sb")
        for qtr in range(4):
            sl = slice(256 * qtr, 256 * qtr + 256)
            nc.scalar.activation(out=pt_sb[:, sl], in_=pt_ps[:, sl],
                                 func=COPY, scale=rs_col[:], alpha=0.0)

        # ---------- transpose P^T -> P natural; cast fp8 ----------
        p8 = work.tile([128, HC, 128], f16, tag="p8")
        for hc in range(HC):
            ptp = ps_tp.tile([128, 128], f16, tag="tp",
                             name=f"p_tp{it}_{hc}")
            nc.tensor.matmul(
                ptp[:], pt_sb[:, 128 * hc:128 * hc + 128], id16[:],
                is_transpose=True, skip_group_check=True)
            nc.vector.tensor_copy(p8[:, hc, :], ptp[:])

        # ---------- update = X P via X^T (fp8, strided 16-col weights) ---
        u_ps = ps_big.tile([128, 2, 512], f32, tag="big", name=f"u_ps{it}")
        for g in range(2):
            for hc in range(HC):
                for b_ in range(4):
                    b = 4 * g + b_
                    wcols = p8[:, hc, :].rearrange(
                        "p (j b) -> p b j", b=8)[:, b, :]
                    nc.tensor.matmul(
                        u_ps[32 * b_:32 * b_ + 16, g, :],
                        wcols,
                        xt8[:, b, hc, :],
                        start=(hc == 0), stop=(hc == HC - 1),
                        skip_group_check=True,
                        tile_position=(0, 32 * b_))
        u_sb = work.tile([128, 2, 512], f32, tag="u_sb")
        for g in range(2):
            for sc in range(SC):
                nc.vector.tensor_copy(
                    u_sb[:, g, 128 * sc:128 * sc + 128],
                    u_ps[:, g, 128 * sc:128 * sc + 128])

        # ---------- transpose update, accumulate logits ----------
        for sc in range(SC):
            for g in range(2):
                utp = ps_tp.tile([128, 128], f32, tag="tp",
                                 name=f"ut_tp{it}_{sc}_{g}")
                nc.tensor.matmul(
                    utp[:], u_sb[:, g, 128 * sc:128 * sc + 128], id32[:],
                    is_transpose=True, skip_group_check=True)
                src = utp.rearrange("p (b j) -> p b j", j=32)
                nc.vector.tensor_add(
                    logits[:, 4 * g:4 * g + 4, sc, :],
                    logits[:, 4 * g:4 * g + 4, sc, :], src[:, :, 0:16])

        # ---------- softmax over capsules -> cpad ----------
        for sc in range(SC):
            ex = small.tile([128, BL, 16], f32, tag="ex")
            nc.scalar.activation(out=ex[:], in_=logits[:, :, sc, :],
                                 func=EXP, scale=1.0, alpha=0.0)
            sm = small.tile([128, BL, 1], f32, tag="sm")
            nc.vector.reduce_sum(sm[:], ex[:], axis=mybir.AxisListType.X)
            rc = small.tile([128, BL, 1], f32, tag="rc")
            nc.vector.reciprocal(rc[:], sm[:])
            nc.vector.tensor_mul(cpad[:, :, sc, 0:16], ex[:],
                                 rc.broadcast_to([128, BL, 16]))

    # ---------- out[b, j, d] = v32[q = j*8 + b, d] via DRAM gather ------
    nc.sync.dma_start(out=scr_d[:], in_=v32[:])
    gather_in = bass.AP(tensor=scr_d.tensor, offset=0,
                        ap=[[512, 16], [64, 8], [1, 64]])
    gather_out = bass.AP(tensor=out_d.tensor, offset=0,
                         ap=[[64, 16], [1024, 8], [1, 64]])
    nc.scalar.dma_start(out=gather_out, in_=gather_in)
    ctx.close()


_CACHE = {}


def _host_consts():
    cb16 = np.zeros((128, NC16), np.float16)
    cb16[:, C_ID16:C_ID16 + 128] = np.eye(128, dtype=np.float16)
    cb16[:, C_ONE] = 1.0                      # onescol
    cb32 = np.zeros((128, NC32), np.float32)
    cb32[:, C_ID32:C_ID32 + 128] = np.eye(128, dtype=np.float32)
    cb32[0, C_ONES1] = 1.0
    cpad = np.zeros((128, BL, SC, 32), np.float16)
    cpad[:, :, :, 0:16] = 1.0 / NCAP          # iteration-0 softmax is exact
    logi = np.zeros((128, BL, SC, 16), np.float32)
    vblk = np.zeros((128, OC, 128), np.float16)
    return {"cb16": cb16, "cb32": cb32, "cpadi": cpad, "logi": logi,
            "vblki": vblk}


def _get_nc():
    if "nc" not in _CACHE:
        nc = bacc.Bacc("TRN2", target_bir_lowering=False, debug=False)
        x_d = nc.dram_tensor("xh", [128, BL, SC, 1024], f16,
                             kind="ExternalInput")
        xt_d = nc.dram_tensor("xth", [128, BL, HC, 512], f16,
                              kind="ExternalInput")
        w_d = nc.dram_tensor("wh", [128, HC, 1024], f16,
                             kind="ExternalInput")
        cb16_d = nc.dram_tensor("cb16", [128, NC16], f16,
                                kind="ExternalInput")
        cb32_d = nc.dram_tensor("cb32", [128, NC32], f32,
                                kind="ExternalInput")
        cpad_d = nc.dram_tensor("cpadi", [128, BL, SC, 32], f16,
                                kind="ExternalInput")
        logits_d = nc.dram_tensor("logi", [128, BL, SC, 16], f32,
                                  kind="ExternalInput")
        vblk_d = nc.dram_tensor("vblki", [128, OC, 128], f16,
                                kind="ExternalInput")
        scr_d = nc.dram_tensor("scratch", [128, 64], f32, kind="Internal")
        out_d = nc.dram_tensor("out", [BL, NCAP, DCAP], f32,
                               kind="ExternalOutput")
        with tile.TileContext(nc) as tc:
            _build_kernel(tc, out_d.ap(), x_d.ap(), xt_d.ap(), w_d.ap(),
                          cb16_d.ap(), cb32_d.ap(), scr_d.ap(),
                          cpad_d.ap(), logits_d.ap(), vblk_d.ap())
        nc.compile()
        _CACHE["nc"] = nc
    return _CACHE["nc"]


def kernel(inputs: np.ndarray, W: np.ndarray, _trace: bool = False):
    """inputs: (512, 64, 1024) f32; W: (1, 1024, 1024) f32.
    Returns (64, 16, 64) f32."""
    nc = _get_nc()
    consts = _host_consts()
    w0 = W[0].astype(np.float16)
    wh = np.ascontiguousarray(w0.reshape(HC, 128, 1024).transpose(1, 0, 2))
    xf = inputs.astype(np.float16)              # (512, 64, 1024)
    in_maps = []
    for c in range(N_CORES):
        xs = xf[:, c * BL:(c + 1) * BL, :]      # (512, BL, 1024)
        xh = np.ascontiguousarray(
            xs.reshape(SC, 128, BL, 1024).transpose(1, 2, 0, 3))
        xth = np.ascontiguousarray(
            xs.reshape(512, BL, HC, 128).transpose(3, 1, 2, 0))
        m = {"xh": xh, "xth": xth, "wh": wh}
        m.update(consts)
        in_maps.append(m)
    kw = {}
    if _trace:
        kw = dict(trace=True, trace_cores=list(range(N_CORES)),
                  stitch_traces=False)
    res = run_bass_kernel_spmd(nc, in_maps, core_ids=list(range(N_CORES)),
                               **kw)
    out = np.concatenate([res.results[c]["out"] for c in range(N_CORES)],
                         axis=0)
    if _trace:
        return out.astype(np.float32), res
    return out.astype(np.float32)
